# revision 4
# baseline (speedup 1.0000x reference)
"""Trainium2 Bass kernel for MinibatchDiscrimination.

Reference op:
    h = (x @ w).reshape(B, U, O)                      # B=512, U=32, O=32
    D[i, o, j] = sum_u |h[i,u,o] - h[j,u,o]|          # pairwise L1 over units
    out[i, o]  = sum_j exp(-D[i,o,j])

Two SPMD launches over 8 NeuronCores:

Launch 1 (h = x @ w, uo-sharded): core c computes hT rows [128c, 128c+128)
for all B columns, in fp8 (x and 16*w cast to e4m3; the PSUM result is
scaled back by 1/16 in the copy-out). fp8 noise perturbs the pairwise L1
distances by <<1, far below the exp(-D) noise floor (min D ~ 19 on this
input regime), and halves the input DMA bytes.

Launch 2 (pairwise phase, data-parallel over query rows, half-pair
windows): each core owns 64 queries and compares each against the 256
columns [i+1, i+256] of the rolled local frame; the diagonal exp(0)=1 and
the transposed-side contributions are folded on the host from the raw exp
tiles (eall output).

Per (query, chunk-of-128-uo-rows) one elementwise op + one PE matmul:
  - DVE chunks (m in 0..4 always; m=5 for q%4!=3) compute
        a = max(h_j, h_i) - S_m[o, i]/8
    in ONE dual-op tensor_scalar (max, subtract). The second scalar
    pre-pays the query-side -S_i term of |d| = 2max - a - b, so the exp
    needs NO bias.
  - ACT chunks (m in 6,7; m=5 for q%4==3) compute a = 0.5*|h_j - h_i| via
    Abs(scale=0.5, bias=-h_i/2) - exact, no S terms.
  - All chunk matmuls share ONE stationary sel2 (2 at p%32==o), so PE
    weight reloads are stripped; the comparison-side -S_j term rides a
    per-query chain-starting matmul over Sq4 (rows 0:32 = -S6/2 read via
    selq6 for q%4!=3 queries, rows 32:64 = -S5/2 via selq5 for q%4==3,
    where S6/S5 sum h over that query type's DVE chunk set).
  - exp(-PSUM) with accum_out gives the row sums (frow); raw exp tiles
    stream to HBM (eall) and the host does the transposed fold.

The DVE/ACT chunk split (5.75 / 2.25) balances measured engine rates:
DVE dual-op [128,256] ~196ns, ACT Abs ~399ns, PE 256-col matmul ~109ns.
"""

import os
import sys

import numpy as np

for _p in ("/opt/trn_rl_repo", "/root/.axon_site/_ro/trn_rl_repo"):
    if os.path.isdir(_p) and _p not in sys.path:
        sys.path.insert(0, _p)

import ml_dtypes  # noqa: E402

B = 512  # batch
D = 2048  # in features
U = 32  # units
O = 32  # units_out
UO = U * O  # 1024
NCORES = 8
BL = B // NCORES  # 64 own queries per core
W = 256  # comparison window width (half of B)

KCH = D // 128  # 16 k-chunks
MCH = UO // 128  # 8 uo-chunks
NQ = 4  # queries batched per PSUM bank via PE column-quadrant matmuls
NG = BL // NQ  # 16 quad groups

ACT_FIX = (6, 7)  # chunks always on ACT (abs form)
# chunk 5 goes to ACT for queries with i % 4 == ACT5_Q, else DVE
ACT5_Q = 3

_CACHE = {}
LAST_RESULTS = None  # BassKernelResults of the most recent run (for profiling)


def _build_h():
    """Launch-1 program: core c computes hT rows [128c, 128c+128) in bf16.

    Inputs are fp8 (x, 16*w); the copy-out applies the 1/16 rescale.
    The PSUM chain is split into two 256-column halves so the first
    half's copy-out and DMA overlap the second half's matmuls.
    """
    if "nc_h" in _CACHE:
        return _CACHE["nc_h"]

    from contextlib import ExitStack

    import concourse.mybir as mybir
    import concourse.tile as tile
    from concourse import bacc

    fp8 = mybir.dt.float8e4
    bf16 = mybir.dt.bfloat16
    f32 = mybir.dt.float32

    nc = bacc.Bacc(
        "TRN2", target_bir_lowering=False, debug=False, enable_asserts=False
    )
    xt_d = nc.dram_tensor("xt", [D, B], fp8, kind="ExternalInput")
    ws_d = nc.dram_tensor("ws", [D, 128], fp8, kind="ExternalInput")
    hts_d = nc.dram_tensor("hts", [128, B], bf16, kind="ExternalOutput")

    with tile.TileContext(nc) as tc, ExitStack() as ctx:
        pool = ctx.enter_context(tc.tile_pool(name="p", bufs=1))
        psum = ctx.enter_context(tc.tile_pool(name="ps", bufs=1, space="PSUM"))
        # strided DMAs split into k-group slabs so they ride parallel
        # DMA queues and matmuls start after the first slab
        KG = 2
        xt_sb = pool.tile([128, KCH * B], fp8, tag="xt")
        ws_sb = pool.tile([128, KCH * 128], fp8, tag="ws")
        xr = xt_sb.rearrange("p (k j) -> p k j", k=KCH)
        xs = xt_d.rearrange("(k p) j -> p k j", k=KCH)
        wr = ws_sb.rearrange("p (k j) -> p k j", k=KCH)
        wsrc = ws_d.rearrange("(k p) j -> p k j", k=KCH)
        for kg in range(0, KCH, KG):
            nc.sync.dma_start(wr[:, kg : kg + KG, :], wsrc[:, kg : kg + KG, :])
            nc.sync.dma_start(xr[:, kg : kg + KG, :], xs[:, kg : kg + KG, :])
        ph = psum.tile([128, B], f32)
        for half in range(2):
            cols = slice(half * 256, half * 256 + 256)
            for k in range(KCH):
                nc.tensor.matmul(
                    ph[:, cols],
                    ws_sb[:, k * 128 : (k + 1) * 128],
                    xt_sb[:, k * B + half * 256 : k * B + half * 256 + 256],
                    start=(k == 0),
                    stop=(k == KCH - 1),
                )
            hts = pool.tile([128, 256], bf16, tag=f"hts{half}", name=f"hts{half}")
            nc.scalar.activation(
                hts[:], ph[:, cols], mybir.ActivationFunctionType.Copy, scale=0.0625
            )
            nc.sync.dma_start(hts_d[:, cols], hts[:])

    nc.compile()
    _CACHE["nc_h"] = nc
    return nc


def _dve_chunks(i):
    return (0, 1, 2, 3, 4) if i % NQ == ACT5_Q else (0, 1, 2, 3, 4, 5)


def _act_chunks(i):
    return (5, 6, 7) if i % NQ == ACT5_Q else (6, 7)


def _build():
    """Build + compile the launch-2 (pairwise) SPMD program."""
    if "nc" in _CACHE:
        return _CACHE["nc"]

    from contextlib import ExitStack

    import concourse.mybir as mybir
    import concourse.tile as tile
    from concourse import bacc

    bf16 = mybir.dt.bfloat16
    f32 = mybir.dt.float32
    AF = mybir.ActivationFunctionType
    AO = mybir.AluOpType

    nc = bacc.Bacc(
        "TRN2", target_bir_lowering=False, debug=False, enable_asserts=False
    )

    ht_d = nc.dram_tensor("ht", [UO, B], bf16, kind="ExternalInput")
    # sel cols 0:32 = sel1 (1 at p%32==o), 32:64 = sel2 (2 at p%32==o),
    # 64:96 = selq6 (2 at p==o, rows 0:32 only), 96:128 = selq5 (2 at
    # p==32+o, rows 32:64 only)
    sel_d = nc.dram_tensor("sel", [128, 128], bf16, kind="ExternalInput")
    frow_d = nc.dram_tensor("frow", [128, NG], f32, kind="ExternalOutput")
    eall_d = nc.dram_tensor("eall", [128, NG * W], bf16, kind="ExternalOutput")

    with tile.TileContext(nc) as tc, ExitStack() as ctx:
        persist = ctx.enter_context(tc.tile_pool(name="persist", bufs=1))
        a_pool = ctx.enter_context(tc.tile_pool(name="a", bufs=12))
        e_pool = ctx.enter_context(tc.tile_pool(name="e", bufs=4))
        ps_pool = ctx.enter_context(tc.tile_pool(name="ps", bufs=1, space="PSUM"))
        pd_pool = ctx.enter_context(tc.tile_pool(name="pd", bufs=5, space="PSUM"))

        # --- persistent tiles ---
        sel_sb = persist.tile([128, 128], bf16, tag="sel")
        nc.sync.dma_start(sel_sb[:], sel_d[:])
        sel1 = sel_sb[:, 0:O]

        F4 = persist.tile([128, NG], f32, tag="F4")

        # phase 1: load hT (from launch 1), build per-query scalars + S data
        hT_all = persist.tile([128, MCH * B], bf16, tag="hT_all")
        nc.sync.dma_start(
            hT_all.rearrange("p (m j) -> p m j", m=MCH),
            ht_d.rearrange("(m p) j -> p m j", m=MCH),
        )
        hT = [hT_all[:, m * B : (m + 1) * B] for m in range(MCH)]

        # +h_i f32 scalars for DVE max chunks (m 0..5)
        hbP = [
            persist.tile([128, BL], f32, tag=f"hbP{m}", name=f"hbP{m}")
            for m in range(6)
        ]
        for m in range(6):
            nc.scalar.activation(hbP[m][:], hT[m][:, 0:BL], AF.Copy)
        # -h_i/2 f32 bias for ACT abs chunks (m 5, 6, 7)
        hbN = {}
        for m in (5, 6, 7):
            hbN[m] = persist.tile([128, BL], f32, tag=f"hbN{m}", name=f"hbN{m}")
            nc.scalar.activation(hbN[m][:], hT[m][:, 0:BL], AF.Copy, scale=-0.5)

        # S chains: S5 = sum over chunks 0..4, Sc5 = chunk 5 alone
        ps_a = ps_pool.tile([O, B], f32, name="ps_a", tag="ps_a")
        for n, m in enumerate((0, 1, 2, 3, 4)):
            nc.tensor.matmul(ps_a[:], sel1, hT[m][:, 0:B], start=(n == 0), stop=(n == 4))
        ps_c = ps_pool.tile([O, B], f32, name="ps_c", tag="ps_c")
        nc.tensor.matmul(ps_c[:], sel1, hT[5][:, 0:B], start=True, stop=True)

        # Sq4 rows 0:32 = -(S5+Sc5)/2 = -S6/2, rows 32:64 = -S5/2 (bf16 rhs)
        Sq4 = persist.tile([64, B], bf16, tag="Sq4")
        nc.scalar.activation(Sq4[O : 2 * O, :], ps_a[:], AF.Copy, scale=-0.5)
        nc.vector.scalar_tensor_tensor(
            Sq4[0:O, :], ps_c[:], -0.5, Sq4[O : 2 * O, :], AO.mult, AO.add
        )

        # s2 scalars: Sc_m[o, i]/8 for m in 0..5, spread over the 4 u-rows
        ps_s2 = ps_pool.tile([O, 6 * BL], f32, name="ps_s2", tag="ps_s2")
        for m in range(6):
            nc.tensor.matmul(
                ps_s2[:, m * BL : (m + 1) * BL],
                sel1,
                hT[m][:, 0:BL],
                start=True,
                stop=True,
            )
        s2f = persist.tile([O, 6 * BL], f32, tag="s2f")
        nc.scalar.activation(s2f[:], ps_s2[:], AF.Copy, scale=0.125)
        hbs2 = persist.tile([128, 6 * BL], f32, tag="hbs2")
        for k in range(4):
            nc.sync.dma_start(hbs2[O * k : O * (k + 1), :], s2f[:])

        # Dependency gates: every phase-2 matmul reads one of the gated
        # stationary tiles, so no differently-weighted matmul can be
        # scheduled into phase 2 (required for the ldweights strip).
        zc_a = persist.tile([128, 1], f32, tag="zc_a")
        nc.vector.tensor_scalar(zc_a[:], hbs2[:, 0:1], 0.0, None, AO.mult)
        zc_b = persist.tile([64, 1], f32, tag="zc_b")
        nc.vector.tensor_scalar(zc_b[:], Sq4[:, 0:1], 0.0, None, AO.mult)
        sel2_t = persist.tile([128, O], bf16, tag="sel2t")
        nc.vector.tensor_scalar(sel2_t[:], sel_sb[:, O : 2 * O], zc_a[:], None, AO.add)
        selq6_t = persist.tile([64, O], bf16, tag="selq6t")
        nc.vector.tensor_scalar(
            selq6_t[:], sel_sb[0:64, 2 * O : 3 * O], zc_b[:], None, AO.add
        )
        selq5_t = persist.tile([64, O], bf16, tag="selq5t")
        nc.vector.tensor_scalar(
            selq5_t[:], sel_sb[0:64, 3 * O : 4 * O], zc_b[:], None, AO.add
        )

        # --- phase 2: per-query windowed pairwise L1 + exp, 4 queries per
        # PSUM bank via PE column quadrants. The exp for quad g is emitted
        # after quad g+1's chunk work so the cross-engine dependency never
        # blocks the ACT/DVE FIFOs.
        pd_tiles = {}

        def emit_quad(g):
            pd = pd_pool.tile([128, W], f32, name=f"pd{g}", tag="pd")
            pd_tiles[g] = pd
            for q in range(NQ):
                i = NQ * g + q
                lo = i + 1
                # chain-starting -S_j matmul: static rhs, so PE starts the
                # chain without waiting on DVE/ACT chunk producers
                selq = selq5_t if i % NQ == ACT5_Q else selq6_t
                nc.tensor.matmul(
                    pd[O * q : O * (q + 1), :],
                    selq[:],
                    Sq4[:, lo : lo + W],
                    start=True,
                    stop=False,
                    tile_position=(0, O * q),
                )
                dset = _dve_chunks(i)
                aset = _act_chunks(i)
                order = list(dset) + list(aset)
                for m in order:
                    a = a_pool.tile([128, W], bf16, tag="a", name=f"a{g}_{q}_{m}")
                    if m in dset:
                        nc.vector.tensor_scalar(
                            a[:],
                            hT[m][:, lo : lo + W],
                            hbP[m][:, i : i + 1],
                            hbs2[:, m * BL + i : m * BL + i + 1],
                            AO.max,
                            AO.subtract,
                        )
                    else:
                        nc.scalar.activation(
                            a[:],
                            hT[m][:, lo : lo + W],
                            AF.Abs,
                            bias=hbN[m][:, i : i + 1],
                            scale=0.5,
                        )
                    nc.tensor.matmul(
                        pd[O * q : O * (q + 1), :],
                        sel2_t[:],
                        a[:],
                        start=False,
                        stop=(m == order[-1]),
                        tile_position=(0, O * q),
                    )

        def emit_exp(g):
            pd = pd_tiles.pop(g)
            e = e_pool.tile([128, W], bf16, tag="e", name=f"e{g}")
            nc.scalar.activation(
                e[:],
                pd[:],
                AF.Exp,
                scale=-1.0,
                accum_out=F4[:, g : g + 1],
            )
            nc.sync.dma_start(eall_d[:, g * W : (g + 1) * W], e[:])

        for g in range(NG):
            emit_quad(g)
            if g >= 1:
                emit_exp(g - 1)
        emit_exp(NG - 1)

        nc.sync.dma_start(frow_d[:], F4[:])

    nc.compile()
    _strip_redundant_ldweights(nc)
    _CACHE["nc"] = nc
    return nc


def _strip_redundant_ldweights(nc):
    """Drop PE weight reloads whose weights AP matches the already-loaded one.

    The Tile lowering splits every matmul into Ldweights+Matmult. Phase 2
    issues runs of matmuls with the same stationary matrix per PE column
    quadrant; reloading per matmul costs PE time. A reload is removable iff
    it has no semaphore waits/updates and its quadrant (tile_position)
    already holds the identical weights AP; any unrecognized PE instruction
    conservatively invalidates the tracked state.
    """
    import concourse.mybir as mybir

    PE = mybir.EngineType.PE
    keep_state = {"InstMatmult", "InstDrain", "InstEventSemaphore", "InstNop"}
    removed = 0
    for blk in nc.m.functions[0].blocks:
        insts = blk.instructions
        out = []
        loaded = {}  # tile_position -> weights key
        for inst in insts:
            nm = type(inst).__name__
            if nm == "InstLdweights":
                ap = inst.ins[0]
                pos = tuple(inst.tile_position or (0, 0))
                key = (
                    ap.memref,
                    ap.offset,
                    tuple(map(tuple, ap.ap)),
                    str(ap.dtype),
                    inst.is_transpose,
                    inst.perf_mode,
                    tuple(inst.tile_size or ()),
                )
                si = inst.sync_info
                has_sync = si is not None and (
                    list(si.on_wait or []) or list(si.on_update or [])
                )
                if not has_sync and loaded.get(pos) == key:
                    removed += 1
                    continue
                if pos == (0, 0) and (inst.tile_size is None):
                    # full-array load clobbers every quadrant
                    loaded = {}
                loaded[pos] = key
            elif nm not in keep_state and getattr(inst, "engine", None) == PE:
                loaded = {}
            out.append(inst)
        if removed:
            blk.instructions = out
    return removed


def _make_inputs_h(x: np.ndarray, w: np.ndarray):
    fp8 = ml_dtypes.float8_e4m3
    xt = np.ascontiguousarray(x.T).astype(fp8)  # [D, B]
    wb = (16.0 * w).astype(fp8)  # [D, UO] scaled into fp8 normal range
    return [
        {"xt": xt, "ws": np.ascontiguousarray(wb[:, 128 * c : 128 * (c + 1)])}
        for c in range(NCORES)
    ]


def _make_sel():
    sel = np.zeros((128, 128), dtype=ml_dtypes.bfloat16)
    p = np.arange(128)
    sel[p, p % O] = 1  # sel1
    sel[p, O + p % O] = 2  # sel2
    sel[p[0:32], 2 * O + p[0:32]] = 2  # selq6 (rows 0:32)
    sel[p[32:64], 3 * O + (p[32:64] - 32)] = 2  # selq5 (rows 32:64)
    return sel


def _make_inputs_main(ht_global: np.ndarray):
    sel = _make_sel()
    return [
        {"ht": np.ascontiguousarray(np.roll(ht_global, -BL * c, axis=1)), "sel": sel}
        for c in range(NCORES)
    ]


def _assemble(results) -> np.ndarray:
    """Host-side gather: diagonal + row accums + transposed col fold."""
    out = np.ones((B, O), dtype=np.float64)
    for c in range(NCORES):
        frow = np.asarray(results[c]["frow"]).astype(np.float64)  # [128, 16]
        # frow[32q + o, g] = row-sum for query i = 4g + q
        fr = frow.reshape(NQ, O, NG)  # [q, o, g]
        rows = fr.transpose(2, 0, 1).reshape(BL, O)  # [i = 4g+q -> (g, q), o]
        out[BL * c : BL * (c + 1), :] += rows
        # transposed-side fold from the raw exp tiles
        eall = np.asarray(results[c]["eall"]).astype(np.float64)  # [128, NG*W]
        e4 = eall.reshape(NQ, O, NG, W)  # [q, o, g, col]
        fold = np.zeros((O, B), dtype=np.float64)
        for g in range(NG):
            for q in range(NQ):
                i = NQ * g + q
                fold[:, i + 1 : i + 1 + W] += e4[q, :, g, :]
        idx = (np.arange(B) + BL * c) % B
        out[idx, :] += fold.T
    return out.astype(np.float32)


def kernel(x: np.ndarray, w: np.ndarray) -> np.ndarray:
    global LAST_RESULTS
    from concourse.bass_utils import run_bass_kernel_spmd

    nc_h = _build_h()
    nc = _build()
    res_h = run_bass_kernel_spmd(
        nc_h, _make_inputs_h(np.asarray(x), np.asarray(w)), list(range(NCORES))
    )
    ht_global = np.concatenate(
        [np.asarray(res_h.results[c]["hts"]) for c in range(NCORES)], axis=0
    )
    res = run_bass_kernel_spmd(nc, _make_inputs_main(ht_global), list(range(NCORES)))
    LAST_RESULTS = (res_h, res)
    return _assemble(res.results)


def _np_reference(x, w):
    h = (x @ w).reshape(B, U, O)
    diffs = h[:, :, :, None] - np.transpose(h, (1, 2, 0))[None, :, :, :]
    return np.exp(-np.abs(diffs).sum(axis=1)).sum(axis=-1)  # [B, O]


def _sim_core(nc, in_map, outs):
    from concourse.bass_interp import CoreSim

    sim = CoreSim(nc, trace=False)
    for name, arr in in_map.items():
        sim.tensor(name)[:] = arr
    sim.simulate(check_with_hw=False)
    return {o: sim.tensor(o).copy() for o in outs}


if __name__ == "__main__":
    # CoreSim checks of both device programs; SCALE=50 shrinks h so the
    # pairwise terms are O(1) and actually exercise the machinery.
    SCALE = float(os.environ.get("KSIM_SCALE", "50"))
    rng = np.random.default_rng(0)
    x = (rng.normal(size=(B, D)) / SCALE).astype(np.float32)
    w = rng.uniform(-0.05, 0.05, size=(D, UO)).astype(np.float32)

    nc_h = _build_h()
    nc = _build()

    hts = []
    for c, im in enumerate(_make_inputs_h(x, w)):
        hts.append(_sim_core(nc_h, im, ["hts"])["hts"])
    ht_global = np.concatenate(hts, axis=0)
    h_ref = (x @ w).reshape(B, UO).T  # [UO, B]
    h_err = np.abs(ht_global.astype(np.float32) - h_ref).max() / max(
        np.abs(h_ref).max(), 1e-9
    )
    print(f"launch-1 simulated; h rel err (fp8 path): {h_err:.4g}")

    results = []
    for c, im in enumerate(_make_inputs_main(ht_global)):
        results.append(_sim_core(nc, im, ["frow", "eall"]))
        print(f"core {c} simulated")
    got = _assemble(results)

    # isolate phase-2 machinery: numpy reference ON THE SIMULATED ht
    h_sim = ht_global.astype(np.float32).T.reshape(B, U, O)
    diffs = h_sim[:, :, :, None] - np.transpose(h_sim, (1, 2, 0))[None, :, :, :]
    exp_ph2 = np.exp(-np.abs(diffs).sum(axis=1)).sum(axis=-1)
    err2 = np.abs(got - exp_ph2).max() / np.abs(exp_ph2).max()
    print("phase-2 rel err vs numpy-on-simulated-h:", err2)

    expected = _np_reference(x, w)
    err = np.abs(got - expected).max() / np.abs(expected).max()
    print("full-chain rel err vs fp32 numpy reference:", err)
    print(got[:2, :4])
    print(expected[:2, :4])


# revision 6
# speedup vs baseline: 1.2628x; 1.2628x over previous
"""Trainium2 Bass kernel for MinibatchDiscrimination.

Reference op:
    h = (x @ w).reshape(B, U, O)                      # B=512, U=32, O=32
    D[i, o, j] = sum_u |h[i,u,o] - h[j,u,o]|          # pairwise L1 over units
    out[i, o]  = sum_j exp(-D[i,o,j])

Two SPMD launches over 8 NeuronCores:

Launch 1 (h = x @ w, uo-sharded): core c computes hT rows [128c, 128c+128)
for all B columns, in fp8 (x and 16*w cast to e4m3; the copy-out applies
the 1/16 rescale). fp8 noise perturbs the pairwise L1 distances by <<1,
far below the exp(-D) scale (min D ~ 19 in this input regime), and halves
the input DMA bytes. DMA issue instructions cost ~700ns each on an engine
queue, so the 8 slab DMAs are spread across four engine queues.

Launch 2 (pairwise phase, data-parallel over query rows, half-pair
windows): each core owns 64 queries, comparing each against the 256
columns [i+1, i+256] of its rolled local frame. Per (query, chunk of 128
uo-rows): one elementwise op + one PE matmul accumulating into a PSUM
quadrant (4 queries per bank via tile_position):
  - DVE chunks (m=0..5): a = max(h_j, h_i), single-op tensor_scalar
    (~196ns effective for [128,256] bf16).
  - ACT chunks (m=6,7): a = 0.5*|h_j - h_i| via Abs(scale=0.5,
    bias=-h_i/2) (~500ns) - exact, no S-correction terms.
  - All chunk matmuls share ONE stationary sel2 (2 at p%32==o), so PE
    weight reloads are stripped. Via |a-b| = 2max(a,b)-a-b, the PSUM
    needs -S6_j - S6_i correction (S6 = sum of h over chunks 0..5): the
    j-side rides a per-query chain-STARTING matmul over Sq4 = -S6/2
    (K=32, selq6 weights; its data dependency on the phase-1 S chains
    also orders every phase-2 PSUM chain after phase 1, which the
    ldweights strip requires); the i-side is the exp bias.
  - exp(-PSUM+bias) tiles stream raw to HBM (eall); the host folds BOTH
    the row sums and the transposed column sums from them (no frow
    output, no accumulator reads).
"""

import os
import sys

import numpy as np

for _p in ("/opt/trn_rl_repo", "/root/.axon_site/_ro/trn_rl_repo"):
    if os.path.isdir(_p) and _p not in sys.path:
        sys.path.insert(0, _p)

import ml_dtypes  # noqa: E402

B = 512  # batch
D = 2048  # in features
U = 32  # units
O = 32  # units_out
UO = U * O  # 1024
NCORES = 8
BL = B // NCORES  # 64 own queries per core
W = 256  # comparison window width (half of B)

KCH = D // 128  # 16 k-chunks
MCH = UO // 128  # 8 uo-chunks
NQ = 4  # queries batched per PSUM bank via PE column-quadrant matmuls
NG = BL // NQ  # 16 quad groups

DVE_SET = (0, 1, 2, 3, 4, 5)  # chunks on DVE (max form)
ACT_SET = (6, 7)  # chunks on ACT (abs form)

_CACHE = {}
LAST_RESULTS = None  # BassKernelResults of the most recent run (for profiling)


def _build_h():
    """Launch-1 program: core c computes hT rows [128c, 128c+128) in bf16."""
    if "nc_h" in _CACHE:
        return _CACHE["nc_h"]

    from contextlib import ExitStack

    import concourse.mybir as mybir
    import concourse.tile as tile
    from concourse import bacc

    fp8 = mybir.dt.float8e4
    bf16 = mybir.dt.bfloat16
    f32 = mybir.dt.float32

    nc = bacc.Bacc(
        "TRN2", target_bir_lowering=False, debug=False, enable_asserts=False
    )
    xt_d = nc.dram_tensor("xt", [D, B], fp8, kind="ExternalInput")
    ws_d = nc.dram_tensor("ws", [D, 128], fp8, kind="ExternalInput")
    hts_d = nc.dram_tensor("hts", [128, B], bf16, kind="ExternalOutput")

    with tile.TileContext(nc) as tc, ExitStack() as ctx:
        pool = ctx.enter_context(tc.tile_pool(name="p", bufs=1))
        psum = ctx.enter_context(tc.tile_pool(name="ps", bufs=1, space="PSUM"))
        # k-group slab DMAs spread across engine queues: each dma_start
        # costs ~700ns of queue issue time, so serializing 8 on one
        # engine would dominate the launch.
        KG = 4
        xt_sb = pool.tile([128, KCH * B], fp8, tag="xt")
        ws_sb = pool.tile([128, KCH * 128], fp8, tag="ws")
        xr = xt_sb.rearrange("p (k j) -> p k j", k=KCH)
        xs = xt_d.rearrange("(k p) j -> p k j", k=KCH)
        wr = ws_sb.rearrange("p (k j) -> p k j", k=KCH)
        wsrc = ws_d.rearrange("(k p) j -> p k j", k=KCH)
        w_eng = [nc.scalar, nc.scalar, nc.scalar, nc.scalar]
        x_eng = [nc.sync, nc.gpsimd, nc.sync, nc.gpsimd]
        for n, kg in enumerate(range(0, KCH, KG)):
            w_eng[n].dma_start(wr[:, kg : kg + KG, :], wsrc[:, kg : kg + KG, :])
            x_eng[n].dma_start(xr[:, kg : kg + KG, :], xs[:, kg : kg + KG, :])
        ph = psum.tile([128, B], f32)
        for k in range(KCH):
            nc.tensor.matmul(
                ph[:],
                ws_sb[:, k * 128 : (k + 1) * 128],
                xt_sb[:, k * B : (k + 1) * B],
                start=(k == 0),
                stop=(k == KCH - 1),
            )
        hts = pool.tile([128, B], bf16, tag="hts")
        nc.scalar.activation(
            hts[:], ph[:], mybir.ActivationFunctionType.Copy, scale=0.0625
        )
        nc.sync.dma_start(hts_d[:], hts[:])

    nc.compile()
    _CACHE["nc_h"] = nc
    return nc


def _build():
    """Build + compile the launch-2 (pairwise) SPMD program."""
    if "nc" in _CACHE:
        return _CACHE["nc"]

    from contextlib import ExitStack

    import concourse.mybir as mybir
    import concourse.tile as tile
    from concourse import bacc

    bf16 = mybir.dt.bfloat16
    f32 = mybir.dt.float32
    AF = mybir.ActivationFunctionType
    AO = mybir.AluOpType

    nc = bacc.Bacc(
        "TRN2", target_bir_lowering=False, debug=False, enable_asserts=False
    )

    ht_d = nc.dram_tensor("ht", [UO, B], bf16, kind="ExternalInput")
    # sel cols 0:32 = sel1 (1 at p%32==o), 32:64 = sel2 (2 at p%32==o),
    # 64:96 = selq6 (2 at p==o, rows 0:32 only)
    sel_d = nc.dram_tensor("sel", [128, 128], bf16, kind="ExternalInput")
    eall_d = nc.dram_tensor("eall", [128, NG * W], bf16, kind="ExternalOutput")

    with tile.TileContext(nc) as tc, ExitStack() as ctx:
        persist = ctx.enter_context(tc.tile_pool(name="persist", bufs=1))
        a_pool = ctx.enter_context(tc.tile_pool(name="a", bufs=12))
        e_pool = ctx.enter_context(tc.tile_pool(name="e", bufs=4))
        ps_pool = ctx.enter_context(tc.tile_pool(name="ps", bufs=1, space="PSUM"))
        pd_pool = ctx.enter_context(tc.tile_pool(name="pd", bufs=5, space="PSUM"))

        sel_sb = persist.tile([128, 128], bf16, tag="sel")
        nc.sync.dma_start(sel_sb[:], sel_d[:])
        sel1 = sel_sb[:, 0:O]
        sel2_t = sel_sb[:, O : 2 * O]

        # --- phase 1: load hT (from launch 1), build scalars + S data ---
        hT_all = persist.tile([128, MCH * B], bf16, tag="hT_all")
        nc.sync.dma_start(
            hT_all.rearrange("p (m j) -> p m j", m=MCH),
            ht_d.rearrange("(m p) j -> p m j", m=MCH),
        )
        hT = [hT_all[:, m * B : (m + 1) * B] for m in range(MCH)]

        # +h_i f32 scalars for the DVE max chunks
        hbP = [
            persist.tile([128, BL], f32, tag=f"hbP{m}", name=f"hbP{m}")
            for m in DVE_SET
        ]
        for n, m in enumerate(DVE_SET):
            nc.scalar.activation(hbP[n][:], hT[m][:, 0:BL], AF.Copy)
        # -h_i/2 f32 bias for the ACT abs chunks
        hbN = {}
        for m in ACT_SET:
            hbN[m] = persist.tile([128, BL], f32, tag=f"hbN{m}", name=f"hbN{m}")
            nc.scalar.activation(hbN[m][:], hT[m][:, 0:BL], AF.Copy, scale=-0.5)

        # S chains over the DVE chunk set: S5v = sum(chunks 0..4), Sc5 = chunk 5
        ps_a = ps_pool.tile([O, B], f32, name="ps_a", tag="ps_a")
        for n in range(5):
            nc.tensor.matmul(
                ps_a[:], sel1, hT[n][:, 0:B], start=(n == 0), stop=(n == 4)
            )
        ps_c = ps_pool.tile([O, B], f32, name="ps_c", tag="ps_c")
        nc.tensor.matmul(ps_c[:], sel1, hT[5][:, 0:B], start=True, stop=True)

        # Sq4 = -S6/2 in bf16 (rhs of the chain-starting matmuls)
        t_a = persist.tile([O, B], f32, tag="t_a")
        nc.scalar.activation(t_a[:], ps_a[:], AF.Copy, scale=-0.5)
        Sq4 = persist.tile([O, B], bf16, tag="Sq4")
        nc.vector.scalar_tensor_tensor(
            Sq4[:], ps_c[:], -0.5, t_a[:], AO.mult, AO.add
        )

        # exp bias: +S6[o, i] per own query, stacked to the quad layout
        # biasS[32q + o, g] = S6[o, 4g + q]
        Sa = persist.tile([O, BL], f32, tag="Sa")
        nc.scalar.activation(Sa[:], ps_a[:, 0:BL], AF.Copy)
        biasT = persist.tile([O, BL], f32, tag="biasT")
        nc.vector.scalar_tensor_tensor(
            biasT[:], ps_c[:, 0:BL], 1.0, Sa[:], AO.mult, AO.add
        )
        biasS = persist.tile([128, NG], f32, tag="biasS")
        for q in range(NQ):
            nc.gpsimd.dma_start(biasS[O * q : O * (q + 1), :], biasT[:, q::NQ])

        # Gate: selq6_t depends on Sq4 <- ps_a/ps_c <- all phase-1 matmuls.
        # Every phase-2 PSUM chain STARTS with a selq6_t matmul, so no
        # differently-weighted matmul can be scheduled into phase 2
        # (required for the ldweights strip); the sel2 chunk matmuls are
        # ordered after their chain's start by the PSUM accumulation group.
        zc = persist.tile([O, 1], f32, tag="zc")
        nc.vector.tensor_scalar(zc[:], Sq4[:, 0:1], 0.0, None, AO.mult)
        selq6_t = persist.tile([O, O], bf16, tag="selq6t")
        nc.vector.tensor_scalar(
            selq6_t[:], sel_sb[0:O, 2 * O : 3 * O], zc[:], None, AO.add
        )

        # --- phase 2 ---
        pd_tiles = {}

        def emit_quad(g):
            pd = pd_pool.tile([128, W], f32, name=f"pd{g}", tag="pd")
            pd_tiles[g] = pd
            for q in range(NQ):
                i = NQ * g + q
                lo = i + 1
                # chain-starting -S6_j matmul (static rhs: PE starts the
                # chain without waiting on DVE/ACT chunk producers)
                nc.tensor.matmul(
                    pd[O * q : O * (q + 1), :],
                    selq6_t[:],
                    Sq4[:, lo : lo + W],
                    start=True,
                    stop=False,
                    tile_position=(0, O * q),
                )
                for m in DVE_SET:
                    a = a_pool.tile([128, W], bf16, tag="a", name=f"a{g}_{q}_{m}")
                    nc.vector.tensor_scalar(
                        a[:],
                        hT[m][:, lo : lo + W],
                        hbP[m][:, i : i + 1],
                        None,
                        AO.max,
                    )
                    nc.tensor.matmul(
                        pd[O * q : O * (q + 1), :],
                        sel2_t,
                        a[:],
                        start=False,
                        stop=False,
                        tile_position=(0, O * q),
                    )
                for m in ACT_SET:
                    a = a_pool.tile([128, W], bf16, tag="a", name=f"b{g}_{q}_{m}")
                    nc.scalar.activation(
                        a[:],
                        hT[m][:, lo : lo + W],
                        AF.Abs,
                        bias=hbN[m][:, i : i + 1],
                        scale=0.5,
                    )
                    nc.tensor.matmul(
                        pd[O * q : O * (q + 1), :],
                        sel2_t,
                        a[:],
                        start=False,
                        stop=(m == ACT_SET[-1]),
                        tile_position=(0, O * q),
                    )

        def emit_exp(g):
            pd = pd_tiles.pop(g)
            e = e_pool.tile([128, W], bf16, tag="e", name=f"e{g}")
            nc.scalar.activation(
                e[:], pd[:], AF.Exp, bias=biasS[:, g : g + 1], scale=-1.0
            )
            eng = nc.sync if g % 2 == 0 else nc.gpsimd
            eng.dma_start(eall_d[:, g * W : (g + 1) * W], e[:])

        for g in range(NG):
            emit_quad(g)
            if g >= 1:
                emit_exp(g - 1)
        emit_exp(NG - 1)

    nc.compile()
    _strip_redundant_ldweights(nc)
    _CACHE["nc"] = nc
    return nc


def _strip_redundant_ldweights(nc):
    """Drop PE weight reloads whose weights AP matches the already-loaded one.

    The Tile lowering splits every matmul into Ldweights+Matmult. Phase 2
    issues runs of matmuls with the same stationary matrix per PE column
    quadrant; reloading per matmul costs PE time. A reload is removable iff
    it has no semaphore waits/updates and its quadrant (tile_position)
    already holds the identical weights AP; any unrecognized PE instruction
    conservatively invalidates the tracked state.
    """
    import concourse.mybir as mybir

    PE = mybir.EngineType.PE
    keep_state = {"InstMatmult", "InstDrain", "InstEventSemaphore", "InstNop"}
    removed = 0
    for blk in nc.m.functions[0].blocks:
        insts = blk.instructions
        out = []
        loaded = {}  # tile_position -> weights key
        for inst in insts:
            nm = type(inst).__name__
            if nm == "InstLdweights":
                ap = inst.ins[0]
                pos = tuple(inst.tile_position or (0, 0))
                key = (
                    ap.memref,
                    ap.offset,
                    tuple(map(tuple, ap.ap)),
                    str(ap.dtype),
                    inst.is_transpose,
                    inst.perf_mode,
                    tuple(inst.tile_size or ()),
                )
                si = inst.sync_info
                has_sync = si is not None and (
                    list(si.on_wait or []) or list(si.on_update or [])
                )
                if not has_sync and loaded.get(pos) == key:
                    removed += 1
                    continue
                if pos == (0, 0) and (inst.tile_size is None):
                    # full-array load clobbers every quadrant
                    loaded = {}
                loaded[pos] = key
            elif nm not in keep_state and getattr(inst, "engine", None) == PE:
                loaded = {}
            out.append(inst)
        if removed:
            blk.instructions = out
    return removed


def _make_inputs_h(x: np.ndarray, w: np.ndarray):
    fp8 = ml_dtypes.float8_e4m3
    xt = np.ascontiguousarray(x.T).astype(fp8)  # [D, B]
    wb = (16.0 * w).astype(fp8)  # [D, UO] scaled into fp8 normal range
    return [
        {"xt": xt, "ws": np.ascontiguousarray(wb[:, 128 * c : 128 * (c + 1)])}
        for c in range(NCORES)
    ]


def _make_sel():
    sel = np.zeros((128, 128), dtype=ml_dtypes.bfloat16)
    p = np.arange(128)
    sel[p, p % O] = 1  # sel1
    sel[p, O + p % O] = 2  # sel2
    sel[p[0:O], 2 * O + p[0:O]] = 2  # selq6 (rows 0:32)
    return sel


def _make_inputs_main(ht_global: np.ndarray):
    sel = _make_sel()
    return [
        {"ht": np.ascontiguousarray(np.roll(ht_global, -BL * c, axis=1)), "sel": sel}
        for c in range(NCORES)
    ]


def _assemble(results) -> np.ndarray:
    """Host-side gather: diagonal + row sums + transposed col fold."""
    out = np.ones((B, O), dtype=np.float64)
    for c in range(NCORES):
        eall = np.asarray(results[c]["eall"]).astype(np.float64)  # [128, NG*W]
        e4 = eall.reshape(NQ, O, NG, W)  # [q, o, g, col]
        # row side: out[i] += sum_c e
        rows = e4.sum(axis=3).transpose(2, 0, 1).reshape(BL, O)  # [4g+q -> (g,q), o]
        out[BL * c : BL * (c + 1), :] += rows
        # transposed side: window col c of query i contributes to j = i+1+c
        fold = np.zeros((O, B), dtype=np.float64)
        for g in range(NG):
            for q in range(NQ):
                i = NQ * g + q
                fold[:, i + 1 : i + 1 + W] += e4[q, :, g, :]
        idx = (np.arange(B) + BL * c) % B
        out[idx, :] += fold.T
    return out.astype(np.float32)


def kernel(x: np.ndarray, w: np.ndarray) -> np.ndarray:
    global LAST_RESULTS
    from concourse.bass_utils import run_bass_kernel_spmd

    nc_h = _build_h()
    nc = _build()
    res_h = run_bass_kernel_spmd(
        nc_h, _make_inputs_h(np.asarray(x), np.asarray(w)), list(range(NCORES))
    )
    ht_global = np.concatenate(
        [np.asarray(res_h.results[c]["hts"]) for c in range(NCORES)], axis=0
    )
    res = run_bass_kernel_spmd(nc, _make_inputs_main(ht_global), list(range(NCORES)))
    LAST_RESULTS = (res_h, res)
    return _assemble(res.results)


def _np_reference(x, w):
    h = (x @ w).reshape(B, U, O)
    diffs = h[:, :, :, None] - np.transpose(h, (1, 2, 0))[None, :, :, :]
    return np.exp(-np.abs(diffs).sum(axis=1)).sum(axis=-1)  # [B, O]


def _sim_core(nc, in_map, outs):
    from concourse.bass_interp import CoreSim

    sim = CoreSim(nc, trace=False)
    for name, arr in in_map.items():
        sim.tensor(name)[:] = arr
    sim.simulate(check_with_hw=False)
    return {o: sim.tensor(o).copy() for o in outs}


if __name__ == "__main__":
    # CoreSim checks of both device programs; SCALE=50 shrinks h so the
    # pairwise terms are O(1) and actually exercise the machinery.
    SCALE = float(os.environ.get("KSIM_SCALE", "50"))
    rng = np.random.default_rng(0)
    x = (rng.normal(size=(B, D)) / SCALE).astype(np.float32)
    w = rng.uniform(-0.05, 0.05, size=(D, UO)).astype(np.float32)

    nc_h = _build_h()
    nc = _build()

    hts = []
    for c, im in enumerate(_make_inputs_h(x, w)):
        hts.append(_sim_core(nc_h, im, ["hts"])["hts"])
    ht_global = np.concatenate(hts, axis=0)
    h_ref = (x @ w).reshape(B, UO).T  # [UO, B]
    h_err = np.abs(ht_global.astype(np.float32) - h_ref).max() / max(
        np.abs(h_ref).max(), 1e-9
    )
    print(f"launch-1 simulated; h rel err (fp8 path): {h_err:.4g}")

    results = []
    for c, im in enumerate(_make_inputs_main(ht_global)):
        results.append(_sim_core(nc, im, ["eall"]))
        print(f"core {c} simulated")
    got = _assemble(results)

    # isolate phase-2 machinery: numpy reference ON THE SIMULATED ht
    h_sim = ht_global.astype(np.float32).T.reshape(B, U, O)
    diffs = h_sim[:, :, :, None] - np.transpose(h_sim, (1, 2, 0))[None, :, :, :]
    exp_ph2 = np.exp(-np.abs(diffs).sum(axis=1)).sum(axis=-1)
    err2 = np.abs(got - exp_ph2).max() / np.abs(exp_ph2).max()
    print("phase-2 rel err vs numpy-on-simulated-h:", err2)

    expected = _np_reference(x, w)
    err = np.abs(got - expected).max() / np.abs(expected).max()
    print("full-chain rel err vs fp32 numpy reference:", err)
    print(got[:2, :4])
    print(expected[:2, :4])


# revision 8
# speedup vs baseline: 1.3912x; 1.1017x over previous
"""Trainium2 Bass kernel for MinibatchDiscrimination.

Reference op:
    h = (x @ w).reshape(B, U, O)                      # B=512, U=32, O=32
    D[i, o, j] = sum_u |h[i,u,o] - h[j,u,o]|          # pairwise L1 over units
    out[i, o]  = sum_j exp(-D[i,o,j])

Two SPMD launches over 8 NeuronCores:

Launch 1 (h = x @ w, uo-sharded): core c computes hT rows [128c, 128c+128)
for all B columns, in fp8 (x and 16*w cast to e4m3; the copy-out applies
the 1/16 rescale). fp8 noise perturbs the pairwise L1 distances by <<1,
far below the exp(-D) scale (min D ~ 19 in this input regime), and halves
the input DMA bytes. DMA issue instructions cost ~700ns each on an engine
queue, so the 8 slab DMAs are spread across four engine queues.

Launch 2 (pairwise phase, data-parallel over query rows, half-pair
windows): each core owns 64 queries, comparing each against the 256
columns [i+1, i+256] of its rolled local frame. Per (query, chunk of 128
uo-rows): one elementwise op + one PE matmul accumulating into a PSUM
quadrant (4 queries per bank via tile_position):
  - DVE chunks (m=0..5): a = max(h_j, h_i), single-op tensor_scalar
    (~196ns effective for [128,256] bf16).
  - ACT chunks (m=6,7): a = 0.5*|h_j - h_i| via Abs(scale=0.5,
    bias=-h_i/2) (~500ns) - exact, no S-correction terms.
  - All chunk matmuls share ONE stationary sel2 (2 at p%32==o), so PE
    weight reloads are stripped. Via |a-b| = 2max(a,b)-a-b, the PSUM
    needs -S6_j - S6_i correction (S6 = sum of h over chunks 0..5): the
    j-side rides a per-query chain-STARTING matmul over Sq4 = -S6/2
    (K=32, selq6 weights; its data dependency on the phase-1 S chains
    also orders every phase-2 PSUM chain after phase 1, which the
    ldweights strip requires); the i-side is the exp bias.
  - exp(-PSUM+bias) tiles stream raw to HBM (eall); the host folds BOTH
    the row sums and the transposed column sums from them (no frow
    output, no accumulator reads).
"""

import os
import sys

import numpy as np

for _p in ("/opt/trn_rl_repo", "/root/.axon_site/_ro/trn_rl_repo"):
    if os.path.isdir(_p) and _p not in sys.path:
        sys.path.insert(0, _p)

import ml_dtypes  # noqa: E402

B = 512  # batch
D = 2048  # in features
U = 32  # units
O = 32  # units_out
UO = U * O  # 1024
NCORES = 8
BL = B // NCORES  # 64 own queries per core
W = 256  # comparison window width (half of B)

KCH = D // 128  # 16 k-chunks
MCH = UO // 128  # 8 uo-chunks
NQ = 4  # queries batched per PSUM bank via PE column-quadrant matmuls
NG = BL // NQ  # 16 quad groups

DVE_SET = (0, 1, 2, 3, 4, 5)  # chunks on DVE (max form)
ACT_SET = (6, 7)  # chunks on ACT (abs form)

_CACHE = {}
LAST_RESULTS = None  # BassKernelResults of the most recent run (for profiling)


def _build_h():
    """Launch-1 program: core c computes hT rows [128c, 128c+128) in bf16."""
    if "nc_h" in _CACHE:
        return _CACHE["nc_h"]

    from contextlib import ExitStack

    import concourse.mybir as mybir
    import concourse.tile as tile
    from concourse import bacc

    fp8 = mybir.dt.float8e4
    bf16 = mybir.dt.bfloat16
    f32 = mybir.dt.float32

    nc = bacc.Bacc(
        "TRN2", target_bir_lowering=False, debug=False, enable_asserts=False
    )
    xt_d = nc.dram_tensor("xt", [D, B], fp8, kind="ExternalInput")
    ws_d = nc.dram_tensor("ws", [D, 128], fp8, kind="ExternalInput")
    hts_d = nc.dram_tensor("hts", [128, B], bf16, kind="ExternalOutput")

    with tile.TileContext(nc) as tc, ExitStack() as ctx:
        pool = ctx.enter_context(tc.tile_pool(name="p", bufs=1))
        psum = ctx.enter_context(tc.tile_pool(name="ps", bufs=1, space="PSUM"))
        # k-group slab DMAs spread across engine queues: each dma_start
        # costs ~700ns of queue issue time, so serializing 8 on one
        # engine would dominate the launch.
        KG = 4
        xt_sb = pool.tile([128, KCH * B], fp8, tag="xt")
        ws_sb = pool.tile([128, KCH * 128], fp8, tag="ws")
        xr = xt_sb.rearrange("p (k j) -> p k j", k=KCH)
        xs = xt_d.rearrange("(k p) j -> p k j", k=KCH)
        wr = ws_sb.rearrange("p (k j) -> p k j", k=KCH)
        wsrc = ws_d.rearrange("(k p) j -> p k j", k=KCH)
        w_eng = [nc.scalar, nc.scalar, nc.scalar, nc.scalar]
        x_eng = [nc.sync, nc.gpsimd, nc.sync, nc.gpsimd]
        for n, kg in enumerate(range(0, KCH, KG)):
            w_eng[n].dma_start(wr[:, kg : kg + KG, :], wsrc[:, kg : kg + KG, :])
            x_eng[n].dma_start(xr[:, kg : kg + KG, :], xs[:, kg : kg + KG, :])
        ph = psum.tile([128, B], f32)
        for k in range(KCH):
            nc.tensor.matmul(
                ph[:],
                ws_sb[:, k * 128 : (k + 1) * 128],
                xt_sb[:, k * B : (k + 1) * B],
                start=(k == 0),
                stop=(k == KCH - 1),
            )
        hts = pool.tile([128, B], bf16, tag="hts")
        nc.scalar.activation(
            hts[:], ph[:], mybir.ActivationFunctionType.Copy, scale=0.0625
        )
        nc.sync.dma_start(hts_d[:], hts[:])

    nc.compile()
    _CACHE["nc_h"] = nc
    return nc


def _build():
    """Build + compile the launch-2 (pairwise) SPMD program."""
    if "nc" in _CACHE:
        return _CACHE["nc"]

    from contextlib import ExitStack

    import concourse.mybir as mybir
    import concourse.tile as tile
    from concourse import bacc

    bf16 = mybir.dt.bfloat16
    f32 = mybir.dt.float32
    AF = mybir.ActivationFunctionType
    AO = mybir.AluOpType

    nc = bacc.Bacc(
        "TRN2", target_bir_lowering=False, debug=False, enable_asserts=False
    )

    ht_d = nc.dram_tensor("ht", [UO, B], bf16, kind="ExternalInput")
    # sel cols 0:32 = sel1 (1 at p%32==o), 32:64 = sel2 (2 at p%32==o),
    # 64:96 = selq6 (2 at p==o, rows 0:32 only)
    sel_d = nc.dram_tensor("sel", [128, 128], bf16, kind="ExternalInput")
    eall_d = nc.dram_tensor("eall", [128, NG * W], bf16, kind="ExternalOutput")

    with tile.TileContext(nc) as tc, ExitStack() as ctx:
        persist = ctx.enter_context(tc.tile_pool(name="persist", bufs=1))
        a_pool = ctx.enter_context(tc.tile_pool(name="a", bufs=12))
        e_pool = ctx.enter_context(tc.tile_pool(name="e", bufs=4))
        ps_pool = ctx.enter_context(tc.tile_pool(name="ps", bufs=1, space="PSUM"))
        pd_pool = ctx.enter_context(tc.tile_pool(name="pd", bufs=5, space="PSUM"))

        sel_sb = persist.tile([128, 128], bf16, tag="sel")
        nc.sync.dma_start(sel_sb[:], sel_d[:])
        sel1 = sel_sb[:, 0:O]
        sel2_t = sel_sb[:, O : 2 * O]

        # --- phase 1: load hT (from launch 1), build scalars + S data ---
        hT_all = persist.tile([128, MCH * B], bf16, tag="hT_all")
        nc.sync.dma_start(
            hT_all.rearrange("p (m j) -> p m j", m=MCH),
            ht_d.rearrange("(m p) j -> p m j", m=MCH),
        )
        hT = [hT_all[:, m * B : (m + 1) * B] for m in range(MCH)]

        # +h_i f32 scalars for the DVE max chunks
        hbP = [
            persist.tile([128, BL], f32, tag=f"hbP{m}", name=f"hbP{m}")
            for m in DVE_SET
        ]
        for n, m in enumerate(DVE_SET):
            nc.scalar.activation(hbP[n][:], hT[m][:, 0:BL], AF.Copy)
        # -h_i/2 f32 bias for the ACT abs chunks
        hbN = {}
        for m in ACT_SET:
            hbN[m] = persist.tile([128, BL], f32, tag=f"hbN{m}", name=f"hbN{m}")
            nc.scalar.activation(hbN[m][:], hT[m][:, 0:BL], AF.Copy, scale=-0.5)

        # S chains over the DVE chunk set: S5v = sum(chunks 0..4), Sc5 = chunk 5
        ps_a = ps_pool.tile([O, B], f32, name="ps_a", tag="ps_a")
        for n in range(5):
            nc.tensor.matmul(
                ps_a[:], sel1, hT[n][:, 0:B], start=(n == 0), stop=(n == 4)
            )
        ps_c = ps_pool.tile([O, B], f32, name="ps_c", tag="ps_c")
        nc.tensor.matmul(ps_c[:], sel1, hT[5][:, 0:B], start=True, stop=True)

        # Sq4 rows 0:32 = -S6/2 (the sel2-weighted chain-start doubles it
        # to the needed -S6); rows 32:128 zero so the same sel2 stationary serves
        # the chain-start matmul and every chunk matmul (single ldweights)
        t_a = persist.tile([O, B], f32, tag="t_a")
        nc.scalar.activation(t_a[:], ps_a[:], AF.Copy, scale=-0.5)
        Sq4 = persist.tile([128, B], bf16, tag="Sq4")
        nc.gpsimd.memset(Sq4[:], 0.0)
        nc.vector.scalar_tensor_tensor(
            Sq4[0:O, :], ps_c[:], -0.5, t_a[:], AO.mult, AO.add
        )

        # exp bias: +S6[o, i] per own query, stacked to the quad layout
        # biasS[32q + o, g] = S6[o, 4g + q]
        Sa = persist.tile([O, BL], f32, tag="Sa")
        nc.scalar.activation(Sa[:], ps_a[:, 0:BL], AF.Copy)
        biasT = persist.tile([O, BL], f32, tag="biasT")
        nc.vector.scalar_tensor_tensor(
            biasT[:], ps_c[:, 0:BL], 1.0, Sa[:], AO.mult, AO.add
        )
        biasS = persist.tile([128, NG], f32, tag="biasS")
        for q in range(NQ):
            nc.gpsimd.dma_start(biasS[O * q : O * (q + 1), :], biasT[:, q::NQ])

        # Ordering: every phase-2 PSUM chain STARTS with the Sq4 matmul,
        # whose rhs depends on ps_a/ps_c <- all phase-1 (sel1) matmuls, so
        # no sel1-weighted matmul can interleave into phase 2 and the
        # ldweights strip reduces phase 2 to a single sel2 load.

        # --- phase 2 ---
        pd_tiles = {}

        def emit_quad(g):
            pd = pd_pool.tile([128, W], f32, name=f"pd{g}", tag="pd")
            pd_tiles[g] = pd
            for q in range(NQ):
                i = NQ * g + q
                lo = i + 1
                # chain-starting -S6_j matmul (static rhs: PE starts the
                # chain without waiting on DVE/ACT chunk producers)
                nc.tensor.matmul(
                    pd[O * q : O * (q + 1), :],
                    sel2_t,
                    Sq4[:, lo : lo + W],
                    start=True,
                    stop=False,
                    tile_position=(0, O * q),
                )
                for m in DVE_SET:
                    a = a_pool.tile([128, W], bf16, tag="a", name=f"a{g}_{q}_{m}")
                    nc.vector.tensor_scalar(
                        a[:],
                        hT[m][:, lo : lo + W],
                        hbP[m][:, i : i + 1],
                        None,
                        AO.max,
                    )
                    nc.tensor.matmul(
                        pd[O * q : O * (q + 1), :],
                        sel2_t,
                        a[:],
                        start=False,
                        stop=False,
                        tile_position=(0, O * q),
                    )
                for m in ACT_SET:
                    a = a_pool.tile([128, W], bf16, tag="a", name=f"b{g}_{q}_{m}")
                    nc.scalar.activation(
                        a[:],
                        hT[m][:, lo : lo + W],
                        AF.Abs,
                        bias=hbN[m][:, i : i + 1],
                        scale=0.5,
                    )
                    nc.tensor.matmul(
                        pd[O * q : O * (q + 1), :],
                        sel2_t,
                        a[:],
                        start=False,
                        stop=(m == ACT_SET[-1]),
                        tile_position=(0, O * q),
                    )

        def emit_exp(g):
            pd = pd_tiles.pop(g)
            e = e_pool.tile([128, W], bf16, tag="e", name=f"e{g}")
            nc.scalar.activation(
                e[:], pd[:], AF.Exp, bias=biasS[:, g : g + 1], scale=-1.0
            )
            eng = nc.sync if g % 2 == 0 else nc.gpsimd
            eng.dma_start(eall_d[:, g * W : (g + 1) * W], e[:])

        for g in range(NG):
            emit_quad(g)
            if g >= 1:
                emit_exp(g - 1)
        emit_exp(NG - 1)

    nc.compile()
    _strip_redundant_ldweights(nc)
    _CACHE["nc"] = nc
    return nc


def _strip_redundant_ldweights(nc):
    """Drop PE weight reloads whose weights AP matches the already-loaded one.

    The Tile lowering splits every matmul into Ldweights+Matmult. Phase 2
    issues runs of matmuls with the same stationary matrix per PE column
    quadrant; reloading per matmul costs PE time. A reload is removable iff
    it has no semaphore waits/updates and its quadrant (tile_position)
    already holds the identical weights AP; any unrecognized PE instruction
    conservatively invalidates the tracked state.
    """
    import concourse.mybir as mybir

    PE = mybir.EngineType.PE
    keep_state = {"InstMatmult", "InstDrain", "InstEventSemaphore", "InstNop"}
    removed = 0
    for blk in nc.m.functions[0].blocks:
        insts = blk.instructions
        out = []
        loaded = {}  # tile_position -> weights key
        for inst in insts:
            nm = type(inst).__name__
            if nm == "InstLdweights":
                ap = inst.ins[0]
                pos = tuple(inst.tile_position or (0, 0))
                key = (
                    ap.memref,
                    ap.offset,
                    tuple(map(tuple, ap.ap)),
                    str(ap.dtype),
                    inst.is_transpose,
                    inst.perf_mode,
                    tuple(inst.tile_size or ()),
                )
                si = inst.sync_info
                has_sync = si is not None and (
                    list(si.on_wait or []) or list(si.on_update or [])
                )
                if not has_sync and loaded.get(pos) == key:
                    removed += 1
                    continue
                if pos == (0, 0) and (inst.tile_size is None):
                    # full-array load clobbers every quadrant
                    loaded = {}
                loaded[pos] = key
            elif nm not in keep_state and getattr(inst, "engine", None) == PE:
                loaded = {}
            out.append(inst)
        if removed:
            blk.instructions = out
    return removed


def _make_inputs_h(x: np.ndarray, w: np.ndarray):
    fp8 = ml_dtypes.float8_e4m3
    xt = np.ascontiguousarray(x.T).astype(fp8)  # [D, B]
    wb = (16.0 * w).astype(fp8)  # [D, UO] scaled into fp8 normal range
    return [
        {"xt": xt, "ws": np.ascontiguousarray(wb[:, 128 * c : 128 * (c + 1)])}
        for c in range(NCORES)
    ]


def _make_sel():
    sel = np.zeros((128, 128), dtype=ml_dtypes.bfloat16)
    p = np.arange(128)
    sel[p, p % O] = 1  # sel1
    sel[p, O + p % O] = 2  # sel2
    sel[p[0:O], 2 * O + p[0:O]] = 2  # selq6 (rows 0:32)
    return sel


def _make_inputs_main(ht_global: np.ndarray):
    sel = _make_sel()
    return [
        {"ht": np.ascontiguousarray(np.roll(ht_global, -BL * c, axis=1)), "sel": sel}
        for c in range(NCORES)
    ]


def _assemble(results) -> np.ndarray:
    """Host-side gather: diagonal + row sums + transposed col fold."""
    out = np.ones((B, O), dtype=np.float64)
    for c in range(NCORES):
        eall = np.asarray(results[c]["eall"]).astype(np.float64)  # [128, NG*W]
        e4 = eall.reshape(NQ, O, NG, W)  # [q, o, g, col]
        # row side: out[i] += sum_c e
        rows = e4.sum(axis=3).transpose(2, 0, 1).reshape(BL, O)  # [4g+q -> (g,q), o]
        out[BL * c : BL * (c + 1), :] += rows
        # transposed side: window col c of query i contributes to j = i+1+c
        fold = np.zeros((O, B), dtype=np.float64)
        for g in range(NG):
            for q in range(NQ):
                i = NQ * g + q
                fold[:, i + 1 : i + 1 + W] += e4[q, :, g, :]
        idx = (np.arange(B) + BL * c) % B
        out[idx, :] += fold.T
    return out.astype(np.float32)


def kernel(x: np.ndarray, w: np.ndarray) -> np.ndarray:
    global LAST_RESULTS
    from concourse.bass_utils import run_bass_kernel_spmd

    nc_h = _build_h()
    nc = _build()
    res_h = run_bass_kernel_spmd(
        nc_h, _make_inputs_h(np.asarray(x), np.asarray(w)), list(range(NCORES))
    )
    ht_global = np.concatenate(
        [np.asarray(res_h.results[c]["hts"]) for c in range(NCORES)], axis=0
    )
    res = run_bass_kernel_spmd(nc, _make_inputs_main(ht_global), list(range(NCORES)))
    LAST_RESULTS = (res_h, res)
    return _assemble(res.results)


def _np_reference(x, w):
    h = (x @ w).reshape(B, U, O)
    diffs = h[:, :, :, None] - np.transpose(h, (1, 2, 0))[None, :, :, :]
    return np.exp(-np.abs(diffs).sum(axis=1)).sum(axis=-1)  # [B, O]


def _sim_core(nc, in_map, outs):
    from concourse.bass_interp import CoreSim

    sim = CoreSim(nc, trace=False)
    for name, arr in in_map.items():
        sim.tensor(name)[:] = arr
    sim.simulate(check_with_hw=False)
    return {o: sim.tensor(o).copy() for o in outs}


if __name__ == "__main__":
    # CoreSim checks of both device programs; SCALE=50 shrinks h so the
    # pairwise terms are O(1) and actually exercise the machinery.
    SCALE = float(os.environ.get("KSIM_SCALE", "50"))
    rng = np.random.default_rng(0)
    x = (rng.normal(size=(B, D)) / SCALE).astype(np.float32)
    w = rng.uniform(-0.05, 0.05, size=(D, UO)).astype(np.float32)

    nc_h = _build_h()
    nc = _build()

    hts = []
    for c, im in enumerate(_make_inputs_h(x, w)):
        hts.append(_sim_core(nc_h, im, ["hts"])["hts"])
    ht_global = np.concatenate(hts, axis=0)
    h_ref = (x @ w).reshape(B, UO).T  # [UO, B]
    h_err = np.abs(ht_global.astype(np.float32) - h_ref).max() / max(
        np.abs(h_ref).max(), 1e-9
    )
    print(f"launch-1 simulated; h rel err (fp8 path): {h_err:.4g}")

    results = []
    for c, im in enumerate(_make_inputs_main(ht_global)):
        results.append(_sim_core(nc, im, ["eall"]))
        print(f"core {c} simulated")
    got = _assemble(results)

    # isolate phase-2 machinery: numpy reference ON THE SIMULATED ht
    h_sim = ht_global.astype(np.float32).T.reshape(B, U, O)
    diffs = h_sim[:, :, :, None] - np.transpose(h_sim, (1, 2, 0))[None, :, :, :]
    exp_ph2 = np.exp(-np.abs(diffs).sum(axis=1)).sum(axis=-1)
    err2 = np.abs(got - exp_ph2).max() / np.abs(exp_ph2).max()
    print("phase-2 rel err vs numpy-on-simulated-h:", err2)

    expected = _np_reference(x, w)
    err = np.abs(got - expected).max() / np.abs(expected).max()
    print("full-chain rel err vs fp32 numpy reference:", err)
    print(got[:2, :4])
    print(expected[:2, :4])
